# revision 15
# baseline (speedup 1.0000x reference)
"""Trainium2 Bass kernel for nn_InterpretableAttention (B=8, N=4096, DIM=1024).

Math: the reference returns softmax(q @ k^T, axis=-1)[:, 0, :] -- only row 0
of the attention matrix. So per batch b:
    q0       = Wq @ x[b,0] + bq                                  [DIM]
    v        = Wk^T @ q0                                         [DIM]
    scores_m = x[b,m] . v   (+ q0.bk, a constant -> cancels in softmax)
    out[b]   = softmax(scores)                                   [N]
bk never affects the output. The N x N score matrix and the full q/k
projections are never materialized.

Sharding: data-parallel over batch, one batch per NeuronCore (B == 8 cores).
Collectives on this stack cost ~75us for even a 32KB ReduceScatter (ring
algorithm, ~10us/step latency floor), so each core redundantly loads the
full Wq^T / Wk (8MB) and computes its own q0/v locally. The kernel is
HBM-DMA-bound: 16.8MB of x[b] plus 8.4MB of weights per core, streamed
back-to-back on both HWDGE rings (sync + scalar) at ~420GB/s so the 16
SDMA engines never idle.

The big dot products run on the DVE, not the PE: x stays in its natural
[m, d] layout (m on partitions), and one fused tensor_tensor_reduce per
[128, 512] half-tile computes 128 partial scores in a single pass
(multiply + free-axis add-reduce, chained across the two d-halves via the
reduce's initial-value operand). M=1 PE matmuls would pay ~780ns per
(LDWEIGHTS+MATMUL) pair -- ~50us for the same work; the DVE does it in
~25us, fully hidden under the DMA stream. Scores land as [128, 32]
(m = tile*128 + partition), the ideal softmax layout.

Per-core device pipeline (all f32):
  DMA   sync ring:   x0, bq, Wq^T (4MB), x m-groups 0,2,4 (2MB), singles
        scalar ring: Wk (4MB), x m-groups 1,3,5, singles
        (the last two x singles reuse the Wq/Wk SBUF slots; the final 8
        x DMAs are single 512KB tiles so the pipeline tail is fine-grained)
  A) q0^T = x0^T Wq^T + bq as [1,1024] (K=1 bias matmul + 16 accumulating
     [128,1]^T x [128,512] matmuls); PE-transpose to [128,8];
     v^T = q0^T Wk as [1,1024]; GpSimd partition_broadcast -> [128,1024].
  B) 64 chained tensor_tensor_reduce ops on DVE -> scores [128, 32].
  C) softmax: free-axis max, partition_all_reduce(max), exp with fused
     row-sum (ACT accum_out), partition_all_reduce(add), reciprocal,
     scale, one [128,32] DMA out.
"""

from contextlib import ExitStack

import numpy as np

import concourse.bass as bass  # noqa: F401
import concourse.tile as tile
from concourse import bacc, bass_isa, mybir
from concourse.bass_utils import run_bass_kernel_spmd

B, N, DIM = 8, 4096, 1024
P = 128          # partitions
KC = DIM // P    # 8 chunks along d (or e)
MT = 512         # phase-A matmul moving free dim (PSUM f32 bank limit)
NT = N // P      # 32 m-tiles of 128 rows
GT = 4           # m-tiles per big DMA group
NG = 6           # big groups (24 tiles); remaining 8 tiles are single DMAs
F32 = mybir.dt.float32

_program_cache = {}


def _build_program():
    if "nc" in _program_cache:
        return _program_cache["nc"]

    nc = bacc.Bacc(
        "TRN2",
        target_bir_lowering=False,
        debug=False,
        enable_asserts=False,
        num_devices=B,
    )
    xr = nc.dram_tensor("xr", [N, DIM], F32, kind="ExternalInput").ap()
    wqt = nc.dram_tensor("wqt", [DIM, DIM], F32, kind="ExternalInput").ap()
    wk = nc.dram_tensor("wk", [DIM, DIM], F32, kind="ExternalInput").ap()
    x0c = nc.dram_tensor("x0c", [P, KC], F32, kind="ExternalInput").ap()
    bqr = nc.dram_tensor("bqr", [1, DIM], F32, kind="ExternalInput").ap()
    out = nc.dram_tensor("out", [P, NT], F32, kind="ExternalOutput").ap()

    with tile.TileContext(nc) as tc, ExitStack() as ctx:
        sb = ctx.enter_context(tc.tile_pool(name="sb", bufs=1))
        shared = ctx.enter_context(tc.tile_pool(name="shared", bufs=2))
        pa = ctx.enter_context(tc.tile_pool(name="pa", bufs=3, space="PSUM"))

        # ---------------- DMA plan ----------------
        x0s = sb.tile([P, KC], F32)
        nc.sync.dma_start(x0s, x0c)
        bqs = sb.tile([1, DIM], F32)
        nc.sync.dma_start(bqs, bqr)
        # wq_all[p, i, e] = Wq^T[i*128+p, e]; wk_all[p, i, d] = Wk[i*128+p, d]
        wq_all = shared.tile([P, KC, DIM], F32, tag="w")
        nc.sync.dma_start(wq_all, wqt.rearrange("(i p) e -> p i e", p=P))
        wk_all = shared.tile([P, KC, DIM], F32, tag="w")
        nc.scalar.dma_start(wk_all, wk.rearrange("(i p) d -> p i d", p=P))
        # x m-tile groups: [128, GT, DIM]; tile (g, J) holds m-rows
        # (g*GT+J)*128 + p. Last 8 m-tiles are single DMAs for a
        # fine-grained pipeline tail; final two live in the shared pool.
        xgs = []
        for g in range(NG):
            xg = sb.tile([P, GT, DIM], F32, name=f"xg{g}")
            eng = nc.sync if g % 2 == 0 else nc.scalar
            eng.dma_start(
                xg,
                xr[g * GT * P : (g + 1) * GT * P, :].rearrange(
                    "(J p) d -> p J d", p=P
                ),
            )
            xgs.append(xg)
        xss = []
        for s in range(NG * GT, NT):
            if s >= NT - 4:
                xst = shared.tile([P, DIM], F32, name=f"xs{s}", tag="w")
            else:
                xst = sb.tile([P, DIM], F32, name=f"xs{s}")
            eng = nc.sync if s % 2 == 0 else nc.scalar
            eng.dma_start(xst, xr[s * P : (s + 1) * P, :])
            xss.append(xst)

        ones = sb.tile([1, 1], F32)
        nc.gpsimd.memset(ones, 1.0)

        # ---------------- Phase A: q0 and v ----------------
        # q0^T [1, 1024] = x0^T @ Wq^T + bq, two 512-wide PSUM halves.
        q0sb = sb.tile([1, DIM], F32)
        for h in range(2):
            q0p = pa.tile([1, MT], F32, tag="ps")
            # bias first via K=1 matmul: q0p = ones^T @ bq_half
            nc.tensor.matmul(
                q0p,
                ones,
                bqs[:, h * MT : (h + 1) * MT],
                start=True,
                stop=False,
                skip_group_check=True,
            )
            for i in range(KC):
                nc.tensor.matmul(
                    q0p,
                    x0s[:, i : i + 1],
                    wq_all[:, i, h * MT : (h + 1) * MT],
                    start=False,
                    stop=(i == KC - 1),
                    skip_group_check=True,
                )
            nc.vector.tensor_copy(q0sb[:, h * MT : (h + 1) * MT], q0p)

        # transpose q0 -> [128, 8] (e on partitions)
        q0Tp = pa.tile([P, KC], F32, tag="ps")
        for i in range(KC):
            nc.tensor.transpose(
                q0Tp[:, i : i + 1], q0sb[:, i * P : (i + 1) * P], ones
            )
        q0T = sb.tile([P, KC], F32)
        nc.vector.tensor_copy(q0T, q0Tp)

        # v^T [1, 1024] = q0^T @ Wk
        vsb = sb.tile([1, DIM], F32)
        for h in range(2):
            vp = pa.tile([1, MT], F32, tag="ps")
            for i in range(KC):
                nc.tensor.matmul(
                    vp,
                    q0T[:, i : i + 1],
                    wk_all[:, i, h * MT : (h + 1) * MT],
                    start=(i == 0),
                    stop=(i == KC - 1),
                )
            nc.vector.tensor_copy(vsb[:, h * MT : (h + 1) * MT], vp)

        # broadcast v to all partitions for the DVE dot products via a
        # K=1 ones-column matmul on the PE (outer product ones x v^T)
        ones_row = sb.tile([1, P], F32)
        nc.gpsimd.memset(ones_row, 1.0)
        vb = sb.tile([P, DIM], F32)
        for h in range(2):
            vbp = pa.tile([P, MT], F32, tag="ps")
            nc.tensor.matmul(
                vbp,
                ones_row,
                vsb[:, h * MT : (h + 1) * MT],
                start=True,
                stop=True,
            )
            nc.vector.tensor_copy(vb[:, h * MT : (h + 1) * MT], vbp)

        # ---------------- Phase B: scores[m] = x[m] . v ----------------
        # per m-tile: elementwise multiply on GpSimd, free-axis add-reduce on
        # DVE, pipelined through a double-buffered product scratch.
        # (tensor_tensor_reduce would fuse these but crashes TRN2 hardware.)
        scores = sb.tile([P, NT], F32)
        for j in range(NT):
            if j < NG * GT:
                xtj = xgs[j // GT][:, j % GT, :]
            else:
                xtj = xss[j - NG * GT]
            prod = sb.tile([P, DIM], F32, name="prod", bufs=2)
            nc.gpsimd.tensor_tensor(prod, xtj, vb, mybir.AluOpType.mult)
            nc.vector.tensor_reduce(
                scores[:, j : j + 1],
                prod,
                axis=mybir.AxisListType.X,
                op=mybir.AluOpType.add,
            )

        # ---------------- Phase C: softmax over all N (rows x tiles) ----------------
        lmax = sb.tile([P, 1], F32)
        nc.vector.tensor_reduce(
            lmax, scores, axis=mybir.AxisListType.X, op=mybir.AluOpType.max
        )
        gmax = sb.tile([P, 1], F32)
        nc.gpsimd.partition_all_reduce(
            gmax, lmax, channels=P, reduce_op=bass_isa.ReduceOp.max
        )
        ngmax = sb.tile([P, 1], F32)
        nc.vector.tensor_scalar_mul(ngmax, gmax, -1.0)
        esb = sb.tile([P, NT], F32)
        ssum = sb.tile([P, 1], F32)
        nc.scalar.activation(
            esb,
            scores,
            mybir.ActivationFunctionType.Exp,
            bias=ngmax,
            scale=1.0,
            accum_out=ssum,
        )
        tsum = sb.tile([P, 1], F32)
        nc.gpsimd.partition_all_reduce(
            tsum, ssum, channels=P, reduce_op=bass_isa.ReduceOp.add
        )
        rinv = sb.tile([P, 1], F32)
        nc.vector.reciprocal(rinv, tsum)
        osb = sb.tile([P, NT], F32)
        nc.scalar.activation(
            osb, esb, mybir.ActivationFunctionType.Copy, bias=0.0, scale=rinv
        )
        nc.sync.dma_start(out, osb)

    nc.compile()
    _program_cache["nc"] = nc
    return nc


def _make_in_maps(x, Wq, bq, Wk):
    x = np.asarray(x, dtype=np.float32)
    wqt_h = np.ascontiguousarray(np.asarray(Wq, np.float32).T)
    wk_h = np.ascontiguousarray(np.asarray(Wk, np.float32))
    bq_h = np.asarray(bq, np.float32).reshape(1, DIM)
    in_maps = []
    for b in range(B):
        in_maps.append(
            {
                "xr": np.ascontiguousarray(x[b]),
                "wqt": wqt_h,
                "wk": wk_h,
                "x0c": np.ascontiguousarray(x[b, 0].reshape(KC, P).T),
                "bqr": bq_h,
            }
        )
    return in_maps


def _unpack_out(arr):
    # device out is [128, 32]: arr[p, j] = prob[m = j*128 + p]
    return np.ascontiguousarray(np.asarray(arr).T).reshape(N)


def kernel(x, Wq, bq, Wk, bk):
    nc = _build_program()
    in_maps = _make_in_maps(x, Wq, bq, Wk)
    res = run_bass_kernel_spmd(nc, in_maps, core_ids=list(range(B)))
    outs = [_unpack_out(res.results[b]["out"]) for b in range(B)]
    return np.stack(outs, axis=0).astype(np.float32)


# revision 16
# speedup vs baseline: 1.0003x; 1.0003x over previous
"""Trainium2 Bass kernel for nn_InterpretableAttention (B=8, N=4096, DIM=1024).

Math: the reference returns softmax(q @ k^T, axis=-1)[:, 0, :] -- only row 0
of the attention matrix. So per batch b:
    q0       = Wq @ x[b,0] + bq                                  [DIM]
    v        = Wk^T @ q0                                         [DIM]
    scores_m = x[b,m] . v   (+ q0.bk, a constant -> cancels in softmax)
    out[b]   = softmax(scores)                                   [N]
bk never affects the output. The N x N score matrix and the full q/k
projections are never materialized.

Sharding: data-parallel over batch, one batch per NeuronCore (B == 8 cores).
Collectives on this stack cost ~75us for even a 32KB ReduceScatter (ring
algorithm, ~10us/step latency floor), so each core redundantly loads the
full Wq^T / Wk (8MB) and computes its own q0/v locally. The kernel is
HBM-DMA-bound: 16.8MB of x[b] plus 8.4MB of weights per core, streamed
back-to-back on both HWDGE rings (sync + scalar) at ~420GB/s.

Weights stream in 16 x 512KB chunks through a 4-slot pool so the phase-A
matmuls pipeline with the weight DMAs instead of waiting for a monolithic
4MB transfer, and the slots recycle (weight SBUF footprint: 16KB/partition
instead of 64KB).

The big dot products run on DVE + GpSimd + ACT, not the PE: x stays in its
natural [m, d] layout (m on partitions). Per [128, 1024] m-tile: an
elementwise multiply by the broadcast v (DVE ~1.2us for 20 tiles, GpSimd
~2.3us for 12 tiles, in parallel) and a free-axis add-reduce on ACT via
activation(Copy, accum_out) (~0.9us x 32). All three engines finish their
shares inside the DMA shadow. (M=1 PE matmuls pay ~780ns per tile-pair =
~50us serial; the banned-fast-path tensor_tensor_reduce crashes TRN2.)
Scores land as [128, 32] (m = tile*128 + partition), the ideal softmax
layout.

Per-core device pipeline (all f32):
  DMA   sync ring:   x0, bq, Wq^T/Wk even chunks, x groups 0,2,4, singles
        scalar ring: Wq^T/Wk odd chunks, x groups 1,3,5, singles
  A) q0^T = x0^T Wq^T + bq as [1,1024] (K=1 bias matmuls + per-chunk
     accumulating [128,1]^T x [128,512] matmuls, chunk-paced by the DMA
     stream); PE-transpose to [128,8]; v^T = q0^T Wk likewise;
     PE ones-outer-product broadcast of v to vb [128,1024].
  B) 32 m-tiles: multiply on DVE/GpSimd, add-reduce on ACT -> scores[128,32].
  C) softmax: free-axis max, partition_all_reduce(max), exp with fused
     row-sum (ACT accum_out), partition_all_reduce(add), reciprocal,
     scale, one [128,32] DMA out.
"""

from contextlib import ExitStack

import numpy as np

import concourse.bass as bass  # noqa: F401
import concourse.tile as tile
from concourse import bacc, bass_isa, mybir
from concourse.bass_utils import run_bass_kernel_spmd

B, N, DIM = 8, 4096, 1024
P = 128          # partitions
KC = DIM // P    # 8 chunks along d (or e)
MT = 512         # phase-A matmul moving free dim (PSUM f32 bank limit)
NT = N // P      # 32 m-tiles of 128 rows
GT = 4           # m-tiles per big DMA group
NG = 6           # big groups (24 tiles); remaining 8 tiles are single DMAs
F32 = mybir.dt.float32

_program_cache = {}


def _build_program():
    if "nc" in _program_cache:
        return _program_cache["nc"]

    nc = bacc.Bacc(
        "TRN2",
        target_bir_lowering=False,
        debug=False,
        enable_asserts=False,
        num_devices=B,
    )
    xr = nc.dram_tensor("xr", [N, DIM], F32, kind="ExternalInput").ap()
    wqt = nc.dram_tensor("wqt", [DIM, DIM], F32, kind="ExternalInput").ap()
    wk = nc.dram_tensor("wk", [DIM, DIM], F32, kind="ExternalInput").ap()
    x0c = nc.dram_tensor("x0c", [P, KC], F32, kind="ExternalInput").ap()
    bqr = nc.dram_tensor("bqr", [1, DIM], F32, kind="ExternalInput").ap()
    out = nc.dram_tensor("out", [P, NT], F32, kind="ExternalOutput").ap()

    with tile.TileContext(nc) as tc, ExitStack() as ctx:
        sb = ctx.enter_context(tc.tile_pool(name="sb", bufs=1))
        wpool = ctx.enter_context(tc.tile_pool(name="wpool", bufs=4))
        pa = ctx.enter_context(tc.tile_pool(name="pa", bufs=3, space="PSUM"))

        # ---------------- DMA plan ----------------
        x0s = sb.tile([P, KC], F32)
        nc.sync.dma_start(x0s, x0c)
        bqs = sb.tile([1, DIM], F32)
        nc.sync.dma_start(bqs, bqr)
        # weight chunks through a 4-slot ring: wq_c[i][p, e] = Wq^T[i*128+p, e]
        wq_c, wk_c = [], []
        for i in range(KC):
            wt = wpool.tile([P, DIM], F32, name=f"wq{i}", tag="w")
            eng = nc.sync if i % 2 == 0 else nc.scalar
            eng.dma_start(wt, wqt[i * P : (i + 1) * P, :])
            wq_c.append(wt)
        for i in range(KC):
            wt = wpool.tile([P, DIM], F32, name=f"wk{i}", tag="w")
            eng = nc.sync if i % 2 == 0 else nc.scalar
            eng.dma_start(wt, wk[i * P : (i + 1) * P, :])
            wk_c.append(wt)
        # x m-tile groups [128, GT, DIM]; tile (g, J) holds m-rows
        # (g*GT+J)*128 + p. Last 8 m-tiles are single DMAs so the pipeline
        # tail is fine-grained.
        xgs = []
        for g in range(NG):
            xg = sb.tile([P, GT, DIM], F32, name=f"xg{g}")
            eng = nc.sync if g % 2 == 0 else nc.scalar
            eng.dma_start(
                xg,
                xr[g * GT * P : (g + 1) * GT * P, :].rearrange(
                    "(J p) d -> p J d", p=P
                ),
            )
            xgs.append(xg)
        xss = []
        for s in range(NG * GT, NT):
            xst = sb.tile([P, DIM], F32, name=f"xs{s}")
            eng = nc.sync if s % 2 == 0 else nc.scalar
            eng.dma_start(xst, xr[s * P : (s + 1) * P, :])
            xss.append(xst)

        ones = sb.tile([1, 1], F32)
        nc.gpsimd.memset(ones, 1.0)
        ones_row = sb.tile([1, P], F32)
        nc.gpsimd.memset(ones_row, 1.0)

        # ---------------- Phase A: q0 and v ----------------
        # q0^T [1, 1024] = x0^T @ Wq^T + bq; two 512-wide PSUM halves
        # accumulated chunk-by-chunk as the weight DMAs land.
        q0p = [pa.tile([1, MT], F32, name=f"q0p{h}", tag="ps") for h in range(2)]
        for h in range(2):
            nc.tensor.matmul(
                q0p[h],
                ones,
                bqs[:, h * MT : (h + 1) * MT],
                start=True,
                stop=False,
                skip_group_check=True,
            )
        for i in range(KC):
            for h in range(2):
                nc.tensor.matmul(
                    q0p[h],
                    x0s[:, i : i + 1],
                    wq_c[i][:, h * MT : (h + 1) * MT],
                    start=False,
                    stop=(i == KC - 1),
                    skip_group_check=True,
                )
        q0sb = sb.tile([1, DIM], F32)
        for h in range(2):
            nc.vector.tensor_copy(q0sb[:, h * MT : (h + 1) * MT], q0p[h])

        # transpose q0 -> [128, 8] (e on partitions)
        q0Tp = pa.tile([P, KC], F32, tag="ps")
        for i in range(KC):
            nc.tensor.transpose(
                q0Tp[:, i : i + 1], q0sb[:, i * P : (i + 1) * P], ones
            )
        q0T = sb.tile([P, KC], F32)
        nc.vector.tensor_copy(q0T, q0Tp)

        # v^T [1, 1024] = q0^T @ Wk, chunk-paced like q0
        vp = [pa.tile([1, MT], F32, name=f"vp{h}", tag="ps") for h in range(2)]
        for i in range(KC):
            for h in range(2):
                nc.tensor.matmul(
                    vp[h],
                    q0T[:, i : i + 1],
                    wk_c[i][:, h * MT : (h + 1) * MT],
                    start=(i == 0),
                    stop=(i == KC - 1),
                    skip_group_check=True,
                )
        vsb = sb.tile([1, DIM], F32)
        for h in range(2):
            nc.vector.tensor_copy(vsb[:, h * MT : (h + 1) * MT], vp[h])

        # broadcast v to all partitions via K=1 ones-column outer product
        vb = sb.tile([P, DIM], F32)
        for h in range(2):
            vbp = pa.tile([P, MT], F32, tag="ps")
            nc.tensor.matmul(
                vbp,
                ones_row,
                vsb[:, h * MT : (h + 1) * MT],
                start=True,
                stop=True,
            )
            nc.vector.tensor_copy(vb[:, h * MT : (h + 1) * MT], vbp)

        # ---------------- Phase B: scores[m] = x[m] . v ----------------
        # multiply on DVE (fast) / GpSimd (slower, takes 3 of every 8 tiles),
        # free-axis add-reduce on ACT via activation(Copy, accum_out).
        scores = sb.tile([P, NT], F32)
        actout = sb.tile([P, DIM], F32)
        for j in range(NT):
            if j < NG * GT:
                xtj = xgs[j // GT][:, j % GT, :]
            else:
                xtj = xss[j - NG * GT]
            if j % 8 < 3:
                prod = sb.tile([P, DIM], F32, name="prodg", bufs=2)
                nc.gpsimd.tensor_tensor(prod, xtj, vb, mybir.AluOpType.mult)
            else:
                prod = sb.tile([P, DIM], F32, name="prodv", bufs=2)
                nc.vector.tensor_tensor(prod, xtj, vb, mybir.AluOpType.mult)
            nc.scalar.activation(
                actout,
                prod,
                mybir.ActivationFunctionType.Copy,
                bias=0.0,
                scale=1.0,
                accum_out=scores[:, j : j + 1],
            )

        # ---------------- Phase C: softmax over all N ----------------
        lmax = sb.tile([P, 1], F32)
        nc.vector.tensor_reduce(
            lmax, scores, axis=mybir.AxisListType.X, op=mybir.AluOpType.max
        )
        gmax = sb.tile([P, 1], F32)
        nc.gpsimd.partition_all_reduce(
            gmax, lmax, channels=P, reduce_op=bass_isa.ReduceOp.max
        )
        ngmax = sb.tile([P, 1], F32)
        nc.vector.tensor_scalar_mul(ngmax, gmax, -1.0)
        esb = sb.tile([P, NT], F32)
        ssum = sb.tile([P, 1], F32)
        nc.scalar.activation(
            esb,
            scores,
            mybir.ActivationFunctionType.Exp,
            bias=ngmax,
            scale=1.0,
            accum_out=ssum,
        )
        tsum = sb.tile([P, 1], F32)
        nc.gpsimd.partition_all_reduce(
            tsum, ssum, channels=P, reduce_op=bass_isa.ReduceOp.add
        )
        rinv = sb.tile([P, 1], F32)
        nc.vector.reciprocal(rinv, tsum)
        osb = sb.tile([P, NT], F32)
        nc.scalar.activation(
            osb, esb, mybir.ActivationFunctionType.Copy, bias=0.0, scale=rinv
        )
        nc.sync.dma_start(out, osb)

    nc.compile()
    _program_cache["nc"] = nc
    return nc


def _make_in_maps(x, Wq, bq, Wk):
    x = np.asarray(x, dtype=np.float32)
    wqt_h = np.ascontiguousarray(np.asarray(Wq, np.float32).T)
    wk_h = np.ascontiguousarray(np.asarray(Wk, np.float32))
    bq_h = np.asarray(bq, np.float32).reshape(1, DIM)
    in_maps = []
    for b in range(B):
        in_maps.append(
            {
                "xr": np.ascontiguousarray(x[b]),
                "wqt": wqt_h,
                "wk": wk_h,
                "x0c": np.ascontiguousarray(x[b, 0].reshape(KC, P).T),
                "bqr": bq_h,
            }
        )
    return in_maps


def _unpack_out(arr):
    # device out is [128, 32]: arr[p, j] = prob[m = j*128 + p]
    return np.ascontiguousarray(np.asarray(arr).T).reshape(N)


def kernel(x, Wq, bq, Wk, bk):
    nc = _build_program()
    in_maps = _make_in_maps(x, Wq, bq, Wk)
    res = run_bass_kernel_spmd(nc, in_maps, core_ids=list(range(B)))
    outs = [_unpack_out(res.results[b]["out"]) for b in range(B)]
    return np.stack(outs, axis=0).astype(np.float32)
